# revision 7
# baseline (speedup 1.0000x reference)
"""GuidedAttention kernel for 8x TRN2 NeuronCores (data-parallel over batch).

Math (per batch b):
  neigh_r = neigh.reshape(B, N*T, H)
  X       = neigh_r * tile(node, N)            # [N*T, H]
  s1      = (X @ W.T + b) / sqrt(nn[0])        # [N*T, 8]
  BA      = softmax(s1.T, axis=-1)             # [8, N*T]
  BW      = BA.sum(0).reshape(N, T)

Device strategy (16 batches per core):
  - Host pre-transposes neigh/node so H lands on the SBUF partition dim
    (pure layout change; device HBM traffic is unchanged) and folds the
    1/sqrt(nn) scale into W and b.
  - Per batch: DMA neigh_t [128, 1600] -> DVE broadcast-multiply by
    node_t -> TensorE matmul against Wsc^T [128, 8].
  - Compute-engine APs must start at a 32-aligned partition, so batches
    are processed in groups of 4: batch j of a group writes its s1 rows
    to PSUM partitions 32j..32j+8 via matmul column tiling, one ScalarE
    bias-copy moves the whole [128, 1600] group to SBUF, and a strided
    SBUF->SBUF DMA (DMA has no partition alignment limits) compacts the
    valid rows into the softmax tile S[(b*8+a), pos].
  - One softmax over S [128, 1600] along the free dim, then BW via a 0/1
    selector matmul summing each batch's 8 hop-rows.
  - neigh_r output is a host-side reshape of the input.
"""

import numpy as np
from contextlib import ExitStack

import concourse.bacc as bacc
import concourse.mybir as mybir
import concourse.tile as tile
from concourse.bass_utils import run_bass_kernel_spmd

B, N, T, H, A = 128, 32, 50, 128, 8
NCORES = 8
BL = B // NCORES          # 16 batches per core
P = N * T                 # 1600 positions
G = 4                     # batches per PSUM group (one per 32-strip)
F32 = mybir.dt.float32

# float32r = single-pass matmul, 1 cycle/row vs fp32's 4 (two half-speed
# passes). Incompatible with tile_position column packing, so each batch gets
# its own [32, 2048] PSUM tile and the ScalarE copy lands on a 32-aligned
# stage strip. Flip to F32 if hardware accuracy turns out insufficient.
MM_DT = mybir.dt.float32r

# matmul moving-operand chunks: <=512 fp32 per PSUM bank, bank-aligned.
CHUNKS = [(0, 512), (512, 512), (1024, 512), (1536, 64)]


def _build_nc():
    nc = bacc.Bacc(
        "TRN2", target_bir_lowering=False, debug=False, enable_asserts=False
    )
    neigh_d = nc.dram_tensor("neigh_t", [BL, H, P], F32, kind="ExternalInput")
    node_d = nc.dram_tensor("node_t", [H, BL * T], F32, kind="ExternalInput")
    wsct_d = nc.dram_tensor("wsct", [H, 4 * A], MM_DT, kind="ExternalInput")
    bias_d = nc.dram_tensor("bias128", [128, 1], F32, kind="ExternalInput")
    p16_d = nc.dram_tensor("p16", [128, BL], F32, kind="ExternalInput")
    ba_d = nc.dram_tensor("ba", [BL * A, P], F32, kind="ExternalOutput")
    bw_d = nc.dram_tensor("bw", [BL, P], F32, kind="ExternalOutput")

    with tile.TileContext(nc) as tc, ExitStack() as ctx:
        const = ctx.enter_context(tc.tile_pool(name="const", bufs=1))
        nodet = const.tile([H, BL * T], F32)
        nc.sync.dma_start(nodet[:], node_d[:])
        wsct = const.tile([H, 4 * A], MM_DT)
        nc.sync.dma_start(wsct[:], wsct_d[:])
        bias128 = const.tile([128, 1], F32)
        nc.sync.dma_start(bias128[:], bias_d[:])
        p16 = const.tile([128, BL], F32)
        nc.sync.dma_start(p16[:], p16_d[:])

        spool = ctx.enter_context(tc.tile_pool(name="smax", bufs=1))
        S = spool.tile([128, P], F32)

        with (
            tc.tile_pool(name="nb", bufs=4) as npool,
            tc.tile_pool(name="xt", bufs=3) as xpool,
            tc.tile_pool(name="stage", bufs=2) as stpool,
            tc.tile_pool(name="s1", bufs=2, space="PSUM") as s1pool,
        ):
            for g in range(BL // G):
                stage = stpool.tile([128, P], F32)
                for j in range(G):
                    b = g * G + j
                    nb = npool.tile([H, P], F32)
                    nc.sync.dma_start(nb[:], neigh_d[b])

                    xt = xpool.tile([H, P], MM_DT)
                    node_bc = (
                        nodet[:, b * T : (b + 1) * T]
                        .rearrange("p (o t) -> p o t", o=1)
                        .broadcast_to([H, N, T])
                    )
                    nc.vector.tensor_mul(
                        xt[:].rearrange("p (n t) -> p n t", t=T),
                        nb[:].rearrange("p (n t) -> p n t", t=T),
                        node_bc,
                    )
                    s1 = s1pool.tile([32, 2048], F32)
                    for o, w in CHUNKS:
                        nc.tensor.matmul(
                            s1[:, o : o + w],
                            wsct[:],
                            xt[:, o : o + w],
                            start=True,
                            stop=True,
                        )
                    # rows 8..32 are replicas (wsct columns tiled 4x); the
                    # compaction DMA keeps only rows 0..8.
                    nc.scalar.activation(
                        stage[32 * j : 32 * (j + 1), :],
                        s1[:, 0:P],
                        mybir.ActivationFunctionType.Identity,
                        bias=bias128[0:32, :],
                        scale=1.0,
                    )
                # Compact stage rows {32j + a} -> S rows {32g + 8j + a}.
                for j in range(G):
                    nc.sync.dma_start(
                        S[32 * g + A * j : 32 * g + A * (j + 1), :],
                        stage[32 * j : 32 * j + A, :],
                    )

        stats = ctx.enter_context(tc.tile_pool(name="stats", bufs=1))
        negm = stats.tile([128, 1], F32)
        nc.vector.reduce_max(negm[:], S[:], axis=mybir.AxisListType.X, negate=True)
        E = spool.tile([128, P], F32)
        nc.scalar.activation(
            E[:], S[:], mybir.ActivationFunctionType.Exp, bias=negm[:], scale=1.0
        )
        ssum = stats.tile([128, 1], F32)
        nc.vector.reduce_sum(ssum[:], E[:], axis=mybir.AxisListType.X)
        rs = stats.tile([128, 1], F32)
        nc.vector.reciprocal(rs[:], ssum[:])
        BAt = spool.tile([128, P], F32)
        nc.vector.tensor_scalar_mul(BAt[:], E[:], rs[:])
        nc.sync.dma_start(ba_d[:], BAt[:])

        with tc.tile_pool(name="bwps", bufs=1, space="PSUM") as bwpool:
            bwps = bwpool.tile([BL, 2048], F32)
            for o, w in CHUNKS:
                # full-fp32 matmul: exact summation of the 8 hop rows
                nc.tensor.matmul(
                    bwps[:, o : o + w], p16[:], BAt[:, o : o + w],
                    start=True, stop=True,
                )
            bws = spool.tile([BL, P], F32)
            nc.scalar.copy(bws[:], bwps[:, 0:P])
            nc.sync.dma_start(bw_d[:], bws[:])

    nc.compile()
    return nc


_NC_CACHE = {}


def get_nc():
    if "nc" not in _NC_CACHE:
        _NC_CACHE["nc"] = _build_nc()
    return _NC_CACHE["nc"]


def make_in_maps(node, neigh, neighbors_number, W, bvec):
    node = np.ascontiguousarray(np.asarray(node, dtype=np.float32))
    neigh = np.ascontiguousarray(np.asarray(neigh, dtype=np.float32))
    W = np.asarray(W, dtype=np.float32)
    bvec = np.asarray(bvec, dtype=np.float32)
    nn = np.asarray(neighbors_number)

    rscale = np.float32(1.0) / np.sqrt(np.asarray(nn[0], dtype=np.float32))
    # W^T replicated 4x along columns: every matmul fills its whole
    # 32-row PSUM strip with real data (M is free; cost is N-bound).
    wsct = np.ascontiguousarray(
        np.tile((W * rscale).T.astype(np.float32), (1, 4))
    )  # [H, 4*A]
    bsc = (bvec * rscale).astype(np.float32)
    bias128 = np.ascontiguousarray(np.tile(bsc, 128 // A).reshape(128, 1))
    p16 = np.zeros((128, BL), np.float32)
    for bl in range(BL):
        p16[bl * A : (bl + 1) * A, bl] = 1.0

    in_maps = []
    for c in range(NCORES):
        sl = slice(c * BL, (c + 1) * BL)
        nsh = neigh[sl].reshape(BL, P, H).transpose(0, 2, 1)  # [BL, H, P]
        node_sh = node[sl].transpose(2, 0, 1).reshape(H, BL * T)  # [H, BL*T]
        in_maps.append(
            {
                "neigh_t": np.ascontiguousarray(nsh),
                "node_t": np.ascontiguousarray(node_sh),
                "wsct": wsct,
                "bias128": bias128,
                "p16": p16,
            }
        )
    return in_maps


def run(node, neigh, neighbors_number, W, bvec, trace=False, **trace_kwargs):
    nc = get_nc()
    in_maps = make_in_maps(node, neigh, neighbors_number, W, bvec)
    res = run_bass_kernel_spmd(
        nc, in_maps, list(range(NCORES)), trace=trace, **trace_kwargs
    )
    BA = np.concatenate([r["ba"].reshape(BL, A, P) for r in res.results], 0)
    BW = np.concatenate([r["bw"].reshape(BL, N, T) for r in res.results], 0)
    neigh_full = np.asarray(neigh, dtype=np.float32)
    neigh_r = np.ascontiguousarray(neigh_full.reshape(B, P, H))
    return (BW.astype(np.float32), BA.astype(np.float32), neigh_r), res


def kernel(node_rnn_output, neigh_rnn_output, neighbors_number, W, b):
    outs, _ = run(node_rnn_output, neigh_rnn_output, neighbors_number, W, b)
    return outs


# revision 8
# speedup vs baseline: 1.2058x; 1.2058x over previous
"""GuidedAttention kernel for 8x TRN2 NeuronCores (data-parallel over batch).

Math (per batch b):
  neigh_r = neigh.reshape(B, N*T, H)
  X       = neigh_r * tile(node, N)            # [N*T, H]
  s1      = (X @ W.T + b) / sqrt(nn[0])        # [N*T, 8]
  BA      = softmax(s1.T, axis=-1)             # [8, N*T]
  BW      = BA.sum(0).reshape(N, T)

Device strategy (16 batches per core):
  - Host pre-transposes neigh/node so H lands on the SBUF partition dim
    (pure layout change; device HBM traffic is unchanged) and folds the
    1/sqrt(nn) scale into W and b.
  - Per batch: DMA neigh_t [128, 1600] -> DVE broadcast-multiply by
    node_t -> TensorE matmul against Wsc^T [128, 8].
  - Compute-engine APs must start at a 32-aligned partition, so batches
    are processed in groups of 4: batch j of a group writes its s1 rows
    to PSUM partitions 32j..32j+8 via matmul column tiling, one ScalarE
    bias-copy moves the whole [128, 1600] group to SBUF, and a strided
    SBUF->SBUF DMA (DMA has no partition alignment limits) compacts the
    valid rows into the softmax tile S[(b*8+a), pos].
  - One softmax over S [128, 1600] along the free dim, then BW via a 0/1
    selector matmul summing each batch's 8 hop-rows.
  - neigh_r output is a host-side reshape of the input.
"""

import numpy as np
from contextlib import ExitStack

import concourse.bacc as bacc
import concourse.mybir as mybir
import concourse.tile as tile
from concourse.bass_utils import run_bass_kernel_spmd

B, N, T, H, A = 128, 32, 50, 128, 8
NCORES = 8
BL = B // NCORES          # 16 batches per core
P = N * T                 # 1600 positions
G = 4                     # batches per PSUM group (one per 32-strip)
F32 = mybir.dt.float32

# float32r = single-pass matmul, 1 cycle/row vs fp32's 4 (two half-speed
# passes). Incompatible with tile_position column packing, so each batch gets
# its own [32, 2048] PSUM tile and the ScalarE copy lands on a 32-aligned
# stage strip. Flip to F32 if hardware accuracy turns out insufficient.
MM_DT = mybir.dt.float32r

# matmul moving-operand chunks: <=512 fp32 per PSUM bank, bank-aligned.
CHUNKS = [(0, 512), (512, 512), (1024, 512), (1536, 64)]


def _build_nc():
    nc = bacc.Bacc(
        "TRN2", target_bir_lowering=False, debug=False, enable_asserts=False
    )
    neigh_d = nc.dram_tensor("neigh_t", [BL, H, P], F32, kind="ExternalInput")
    node_d = nc.dram_tensor("node_t", [H, BL * T], F32, kind="ExternalInput")
    wsct_d = nc.dram_tensor("wsct", [H, 4 * A], MM_DT, kind="ExternalInput")
    bias_d = nc.dram_tensor("bias128", [128, 1], F32, kind="ExternalInput")
    p16_d = nc.dram_tensor("p16", [128, BL], F32, kind="ExternalInput")
    ba_d = nc.dram_tensor("ba", [BL * A, P], F32, kind="ExternalOutput")
    bw_d = nc.dram_tensor("bw", [BL, P], F32, kind="ExternalOutput")

    with tile.TileContext(nc) as tc, ExitStack() as ctx:
        const = ctx.enter_context(tc.tile_pool(name="const", bufs=1))
        nodet = const.tile([H, BL * T], F32)
        nc.gpsimd.dma_start(nodet[:], node_d[:])
        wsct = const.tile([H, 4 * A], MM_DT)
        nc.gpsimd.dma_start(wsct[:], wsct_d[:])
        bias128 = const.tile([128, 1], F32)
        nc.gpsimd.dma_start(bias128[:], bias_d[:])
        p16 = const.tile([128, BL], F32)
        nc.gpsimd.dma_start(p16[:], p16_d[:])

        spool = ctx.enter_context(tc.tile_pool(name="smax", bufs=1))
        S = spool.tile([128, P], F32)

        with (
            tc.tile_pool(name="nb", bufs=4) as npool,
            tc.tile_pool(name="xt", bufs=3) as xpool,
            tc.tile_pool(name="stage", bufs=2) as stpool,
            tc.tile_pool(name="s1", bufs=2, space="PSUM") as s1pool,
        ):
            for g in range(BL // G):
                stage = stpool.tile([128, P], F32)
                for j in range(G):
                    b = g * G + j
                    nb = npool.tile([H, P], F32)
                    nc.sync.dma_start(nb[:], neigh_d[b])

                    xt = xpool.tile([H, P], MM_DT)
                    node_bc = (
                        nodet[:, b * T : (b + 1) * T]
                        .rearrange("p (o t) -> p o t", o=1)
                        .broadcast_to([H, N, T])
                    )
                    nc.vector.tensor_mul(
                        xt[:].rearrange("p (n t) -> p n t", t=T),
                        nb[:].rearrange("p (n t) -> p n t", t=T),
                        node_bc,
                    )
                    s1 = s1pool.tile([32, 2048], F32)
                    for o, w in CHUNKS:
                        nc.tensor.matmul(
                            s1[:, o : o + w],
                            wsct[:],
                            xt[:, o : o + w],
                            start=True,
                            stop=True,
                        )
                    # rows 8..32 are replicas (wsct columns tiled 4x); the
                    # compaction DMA keeps only rows 0..8.
                    nc.scalar.activation(
                        stage[32 * j : 32 * (j + 1), :],
                        s1[:, 0:P],
                        mybir.ActivationFunctionType.Identity,
                        bias=bias128[0:32, :],
                        scale=1.0,
                    )
                # Compact stage rows {32j + a} -> S rows {32g + 8j + a}.
                # On the SWDGE queue: the sync HWDGE FIFO stays exclusive to
                # neigh loads, so a compaction waiting on ScalarE can't stall
                # the next group's loads queued behind it.
                for j in range(G):
                    nc.gpsimd.dma_start(
                        S[32 * g + A * j : 32 * g + A * (j + 1), :],
                        stage[32 * j : 32 * j + A, :],
                    )

        stats = ctx.enter_context(tc.tile_pool(name="stats", bufs=1))
        negm = stats.tile([128, 1], F32)
        nc.vector.reduce_max(negm[:], S[:], axis=mybir.AxisListType.X, negate=True)
        E = spool.tile([128, P], F32)
        nc.scalar.activation(
            E[:], S[:], mybir.ActivationFunctionType.Exp, bias=negm[:], scale=1.0
        )
        ssum = stats.tile([128, 1], F32)
        nc.vector.reduce_sum(ssum[:], E[:], axis=mybir.AxisListType.X)
        rs = stats.tile([128, 1], F32)
        nc.vector.reciprocal(rs[:], ssum[:])
        # BA = E * r on ScalarE (per-partition scale) while the BW path runs
        # on DVE+PE directly from E: BW row b = sum_a E[(b,a)] * r[(b,a)]
        # via the selector scaled by r, so it needn't wait for BA.
        BAt = spool.tile([128, P], F32)
        nc.scalar.activation(
            BAt[:], E[:], mybir.ActivationFunctionType.Copy, scale=rs[:]
        )
        nc.sync.dma_start(ba_d[:], BAt[:])

        with tc.tile_pool(name="bwps", bufs=1, space="PSUM") as bwpool:
            p16r = stats.tile([128, BL], F32)
            nc.vector.tensor_scalar_mul(p16r[:], p16[:], rs[:])
            bwps = bwpool.tile([BL, 2048], F32)
            for o, w in CHUNKS:
                # full-fp32 matmul: exact summation of the 8 hop rows
                nc.tensor.matmul(
                    bwps[:, o : o + w], p16r[:], E[:, o : o + w],
                    start=True, stop=True,
                )
            bws = spool.tile([BL, P], F32)
            nc.vector.tensor_copy(bws[:], bwps[:, 0:P])
            nc.gpsimd.dma_start(bw_d[:], bws[:])

    nc.compile()
    return nc


_NC_CACHE = {}


def get_nc():
    if "nc" not in _NC_CACHE:
        _NC_CACHE["nc"] = _build_nc()
    return _NC_CACHE["nc"]


def make_in_maps(node, neigh, neighbors_number, W, bvec):
    node = np.ascontiguousarray(np.asarray(node, dtype=np.float32))
    neigh = np.ascontiguousarray(np.asarray(neigh, dtype=np.float32))
    W = np.asarray(W, dtype=np.float32)
    bvec = np.asarray(bvec, dtype=np.float32)
    nn = np.asarray(neighbors_number)

    rscale = np.float32(1.0) / np.sqrt(np.asarray(nn[0], dtype=np.float32))
    # W^T replicated 4x along columns: every matmul fills its whole
    # 32-row PSUM strip with real data (M is free; cost is N-bound).
    wsct = np.ascontiguousarray(
        np.tile((W * rscale).T.astype(np.float32), (1, 4))
    )  # [H, 4*A]
    bsc = (bvec * rscale).astype(np.float32)
    bias128 = np.ascontiguousarray(np.tile(bsc, 128 // A).reshape(128, 1))
    p16 = np.zeros((128, BL), np.float32)
    for bl in range(BL):
        p16[bl * A : (bl + 1) * A, bl] = 1.0

    in_maps = []
    for c in range(NCORES):
        sl = slice(c * BL, (c + 1) * BL)
        nsh = neigh[sl].reshape(BL, P, H).transpose(0, 2, 1)  # [BL, H, P]
        node_sh = node[sl].transpose(2, 0, 1).reshape(H, BL * T)  # [H, BL*T]
        in_maps.append(
            {
                "neigh_t": np.ascontiguousarray(nsh),
                "node_t": np.ascontiguousarray(node_sh),
                "wsct": wsct,
                "bias128": bias128,
                "p16": p16,
            }
        )
    return in_maps


def run(node, neigh, neighbors_number, W, bvec, trace=False, **trace_kwargs):
    nc = get_nc()
    in_maps = make_in_maps(node, neigh, neighbors_number, W, bvec)
    res = run_bass_kernel_spmd(
        nc, in_maps, list(range(NCORES)), trace=trace, **trace_kwargs
    )
    BA = np.concatenate([r["ba"].reshape(BL, A, P) for r in res.results], 0)
    BW = np.concatenate([r["bw"].reshape(BL, N, T) for r in res.results], 0)
    neigh_full = np.asarray(neigh, dtype=np.float32)
    neigh_r = np.ascontiguousarray(neigh_full.reshape(B, P, H))
    return (BW.astype(np.float32), BA.astype(np.float32), neigh_r), res


def kernel(node_rnn_output, neigh_rnn_output, neighbors_number, W, b):
    outs, _ = run(node_rnn_output, neigh_rnn_output, neighbors_number, W, b)
    return outs


# revision 9
# speedup vs baseline: 1.2610x; 1.0458x over previous
"""GuidedAttention kernel for 8x TRN2 NeuronCores (data-parallel over batch).

Math (per batch b):
  neigh_r = neigh.reshape(B, N*T, H)
  X       = neigh_r * tile(node, N)            # [N*T, H]
  s1      = (X @ W.T + b) / sqrt(nn[0])        # [N*T, 8]
  BA      = softmax(s1.T, axis=-1)             # [8, N*T]
  BW      = BA.sum(0).reshape(N, T)

Device strategy (16 batches per core):
  - Host pre-transposes neigh/node so H lands on the SBUF partition dim
    (pure layout change; device HBM traffic is unchanged) and folds the
    1/sqrt(nn) scale into W and b.
  - Per batch: DMA neigh_t [128, 1600] -> DVE broadcast-multiply by
    node_t -> TensorE matmul against Wsc^T [128, 8].
  - Compute-engine APs must start at a 32-aligned partition, so batches
    are processed in groups of 4: batch j of a group writes its s1 rows
    to PSUM partitions 32j..32j+8 via matmul column tiling, one ScalarE
    bias-copy moves the whole [128, 1600] group to SBUF, and a strided
    SBUF->SBUF DMA (DMA has no partition alignment limits) compacts the
    valid rows into the softmax tile S[(b*8+a), pos].
  - One softmax over S [128, 1600] along the free dim, then BW via a 0/1
    selector matmul summing each batch's 8 hop-rows.
  - neigh_r output is a host-side reshape of the input.
"""

import numpy as np
from contextlib import ExitStack

import concourse.bacc as bacc
import concourse.mybir as mybir
import concourse.tile as tile
from concourse.bass_utils import run_bass_kernel_spmd

B, N, T, H, A = 128, 32, 50, 128, 8
NCORES = 8
BL = B // NCORES          # 16 batches per core
P = N * T                 # 1600 positions
G = 4                     # batches per PSUM group (one per 32-strip)
F32 = mybir.dt.float32

# float32r = single-pass matmul, 1 cycle/row vs fp32's 4 (two half-speed
# passes). Incompatible with tile_position column packing, so each batch gets
# its own [32, 2048] PSUM tile and the ScalarE copy lands on a 32-aligned
# stage strip. Flip to F32 if hardware accuracy turns out insufficient.
MM_DT = mybir.dt.float32r

# matmul moving-operand chunks: <=512 fp32 per PSUM bank, bank-aligned.
CHUNKS = [(0, 512), (512, 512), (1024, 512), (1536, 64)]


def _build_nc():
    nc = bacc.Bacc(
        "TRN2", target_bir_lowering=False, debug=False, enable_asserts=False
    )
    neigh_d = nc.dram_tensor("neigh_t", [BL, H, P], F32, kind="ExternalInput")
    node_d = nc.dram_tensor("node_t", [H, BL * T], F32, kind="ExternalInput")
    wsct_d = nc.dram_tensor("wsct", [H, 4 * A], MM_DT, kind="ExternalInput")
    p16_d = nc.dram_tensor("p16", [128, BL], F32, kind="ExternalInput")
    ba_d = nc.dram_tensor("ba", [BL * A, P], F32, kind="ExternalOutput")
    bw_d = nc.dram_tensor("bw", [BL, P], F32, kind="ExternalOutput")

    with tile.TileContext(nc) as tc, ExitStack() as ctx:
        const = ctx.enter_context(tc.tile_pool(name="const", bufs=1))
        nodet = const.tile([H, BL * T], F32)
        nc.gpsimd.dma_start(nodet[:], node_d[:])
        wsct = const.tile([H, 4 * A], MM_DT)
        nc.gpsimd.dma_start(wsct[:], wsct_d[:])
        p16 = const.tile([128, BL], F32)
        nc.gpsimd.dma_start(p16[:], p16_d[:])

        spool = ctx.enter_context(tc.tile_pool(name="smax", bufs=1))
        S = spool.tile([128, P], F32)

        with (
            tc.tile_pool(name="nb", bufs=BL) as npool,
            tc.tile_pool(name="xt", bufs=3) as xpool,
            tc.tile_pool(name="stage", bufs=2) as stpool,
            tc.tile_pool(name="s1", bufs=2, space="PSUM") as s1pool,
        ):
            nbs = []
            for b in range(BL):
                nb = npool.tile([H, P], F32)
                nc.sync.dma_start(nb[:], neigh_d[b])
                nbs.append(nb)
            for g in range(BL // G):
                stage = stpool.tile([128, P], F32)
                for j in range(G):
                    b = g * G + j
                    nb = nbs[b]

                    xt = xpool.tile([H, P], MM_DT)
                    node_bc = (
                        nodet[:, b * T : (b + 1) * T]
                        .rearrange("p (o t) -> p o t", o=1)
                        .broadcast_to([H, N, T])
                    )
                    nc.vector.tensor_mul(
                        xt[:].rearrange("p (n t) -> p n t", t=T),
                        nb[:].rearrange("p (n t) -> p n t", t=T),
                        node_bc,
                    )
                    s1 = s1pool.tile([32, 2048], F32)
                    for o, w in CHUNKS:
                        nc.tensor.matmul(
                            s1[:, o : o + w],
                            wsct[:],
                            xt[:, o : o + w],
                            start=True,
                            stop=True,
                        )
                    # The bias b[a]/scale is constant along each softmax row, so
                    # it cancels in softmax(s1) and is dropped entirely. Rows
                    # 8..32 are replicas (wsct columns tiled 4x); the
                    # compaction DMA keeps only rows 0..8.
                    nc.scalar.copy(
                        stage[32 * j : 32 * (j + 1), :], s1[:, 0:P]
                    )
                # Compact stage rows {32j + a} -> S rows {32g + 8j + a}.
                # On the SWDGE queue: the sync HWDGE FIFO stays exclusive to
                # neigh loads, so a compaction waiting on ScalarE can't stall
                # the next group's loads queued behind it.
                for j in range(G):
                    nc.gpsimd.dma_start(
                        S[32 * g + A * j : 32 * g + A * (j + 1), :],
                        stage[32 * j : 32 * j + A, :],
                    )

        stats = ctx.enter_context(tc.tile_pool(name="stats", bufs=1))
        negm = stats.tile([128, 1], F32)
        nc.vector.reduce_max(negm[:], S[:], axis=mybir.AxisListType.X, negate=True)
        E = spool.tile([128, P], MM_DT)
        nc.scalar.activation(
            E[:], S[:], mybir.ActivationFunctionType.Exp, bias=negm[:], scale=1.0
        )
        ssum = stats.tile([128, 1], F32)
        nc.vector.reduce_sum(ssum[:], E[:], axis=mybir.AxisListType.X)
        rs = stats.tile([128, 1], F32)
        nc.vector.reciprocal(rs[:], ssum[:])
        # BA = E * r on ScalarE (per-partition scale) while the BW path runs
        # on DVE+PE directly from E: BW row b = sum_a E[(b,a)] * r[(b,a)]
        # via the selector scaled by r, so it needn't wait for BA.
        BAt = spool.tile([128, P], F32)
        nc.scalar.activation(
            BAt[:], E[:], mybir.ActivationFunctionType.Copy, scale=rs[:]
        )
        nc.sync.dma_start(ba_d[:], BAt[:])

        with tc.tile_pool(name="bwps", bufs=1, space="PSUM") as bwpool:
            p16r = stats.tile([128, BL], MM_DT)
            nc.vector.tensor_scalar_mul(p16r[:], p16[:], rs[:])
            bwps = bwpool.tile([BL, 2048], F32)
            for o, w in CHUNKS:
                nc.tensor.matmul(
                    bwps[:, o : o + w], p16r[:], E[:, o : o + w],
                    start=True, stop=True,
                )
            bws = spool.tile([BL, P], F32)
            nc.vector.tensor_copy(bws[:], bwps[:, 0:P])
            nc.gpsimd.dma_start(bw_d[:], bws[:])

    nc.compile()
    return nc


_NC_CACHE = {}


def get_nc():
    if "nc" not in _NC_CACHE:
        _NC_CACHE["nc"] = _build_nc()
    return _NC_CACHE["nc"]


def make_in_maps(node, neigh, neighbors_number, W, bvec):
    node = np.ascontiguousarray(np.asarray(node, dtype=np.float32))
    neigh = np.ascontiguousarray(np.asarray(neigh, dtype=np.float32))
    W = np.asarray(W, dtype=np.float32)
    bvec = np.asarray(bvec, dtype=np.float32)
    nn = np.asarray(neighbors_number)

    rscale = np.float32(1.0) / np.sqrt(np.asarray(nn[0], dtype=np.float32))
    # W^T replicated 4x along columns: every matmul fills its whole
    # 32-row PSUM strip with real data (M is free; cost is N-bound).
    wsct = np.ascontiguousarray(
        np.tile((W * rscale).T.astype(np.float32), (1, 4))
    )  # [H, 4*A]
    p16 = np.zeros((128, BL), np.float32)
    for bl in range(BL):
        p16[bl * A : (bl + 1) * A, bl] = 1.0

    in_maps = []
    for c in range(NCORES):
        sl = slice(c * BL, (c + 1) * BL)
        nsh = neigh[sl].reshape(BL, P, H).transpose(0, 2, 1)  # [BL, H, P]
        node_sh = node[sl].transpose(2, 0, 1).reshape(H, BL * T)  # [H, BL*T]
        in_maps.append(
            {
                "neigh_t": np.ascontiguousarray(nsh),
                "node_t": np.ascontiguousarray(node_sh),
                "wsct": wsct,
                "p16": p16,
            }
        )
    return in_maps


def run(node, neigh, neighbors_number, W, bvec, trace=False, **trace_kwargs):
    nc = get_nc()
    in_maps = make_in_maps(node, neigh, neighbors_number, W, bvec)
    res = run_bass_kernel_spmd(
        nc, in_maps, list(range(NCORES)), trace=trace, **trace_kwargs
    )
    BA = np.concatenate([r["ba"].reshape(BL, A, P) for r in res.results], 0)
    BW = np.concatenate([r["bw"].reshape(BL, N, T) for r in res.results], 0)
    neigh_full = np.asarray(neigh, dtype=np.float32)
    neigh_r = np.ascontiguousarray(neigh_full.reshape(B, P, H))
    return (BW.astype(np.float32), BA.astype(np.float32), neigh_r), res


def kernel(node_rnn_output, neigh_rnn_output, neighbors_number, W, b):
    outs, _ = run(node_rnn_output, neigh_rnn_output, neighbors_number, W, b)
    return outs
